# revision 4
# baseline (speedup 1.0000x reference)
"""Trainium2 Bass kernel for nn_DifferentiableBSpline (Catmull-Rom spline eval).

The reference maps control_points [B, 16, 2] -> trajectory [B, 256, 2] where,
for the fixed schedule (n_cp=16, num_output_points=256), every output point is
a fixed linear combination of the 16 control points of its sample:

    out[b, j, c] = sum_k W[j, k] * cp[b, k, c]

with W[256, 16] folding the Hermite basis, the per-segment t schedule and the
boundary mirroring. On device this is a tiny-K batched matmul, memory bound on
the 128 MB output.

Per-core layout (pure data parallel over batch, B_shard = 8192):
  - groups of 512 batches; input tile [128, (q=4, kc=32)] loaded contiguously
  - DVE StreamTranspose (32x32 blocks) turns it into 4 row-groups of
    lhsT = [kc=32, 128 batches] stacked on partitions
  - 4 TensorE matmuls (K=32 row-tiled at partition 32a, M=128, N=512) against
    a replicated constant W2 [128, 512] -> psum [128, 512] each
  - PSUM drained to SBUF by DVE (a=0,1) and ACT (a=2,3), then one 1 MiB DMA
    writes 512 batches of output.
"""

import numpy as np

import concourse.mybir as mybir
from concourse import bacc
from concourse.tile import TileContext
from concourse.bass_utils import run_bass_kernel_spmd

N_CORES = 8
B_TOTAL = 65536
B_SHARD = B_TOTAL // N_CORES  # 8192
N_CP = 16
T_OUT = 256
GROUP_B = 512
GROUPS = B_SHARD // GROUP_B  # 16


def _spline_weights() -> np.ndarray:
    """W[256, 16]: trajectory[b] = W @ cp[b] (per coordinate)."""
    segments = N_CP - 1
    pps = T_OUT // segments + 1
    seg_list, t_list = [], []
    count = 0
    for i in range(segments):
        if i == segments - 1:
            ts = np.linspace(0.0, 1.0, T_OUT - count)
        else:
            ts = np.linspace(0.0, 1.0, pps)[:-1]
        seg_list.append(np.full(ts.shape, i, dtype=np.int64))
        t_list.append(ts)
        count += len(ts)
    seg = np.concatenate(seg_list)
    t = np.concatenate(t_list).astype(np.float32)
    assert len(seg) == T_OUT

    t2, t3 = t * t, t * t * t
    h00 = 2 * t3 - 3 * t2 + 1
    h10 = t3 - 2 * t2 + t
    h01 = -2 * t3 + 3 * t2
    h11 = t3 - t2

    j = np.arange(T_OUT)
    w_ext = np.zeros((T_OUT, N_CP + 2), dtype=np.float64)
    w_ext[j, seg] += -0.5 * h10
    w_ext[j, seg + 1] += h00 - 0.5 * h11
    w_ext[j, seg + 2] += h01 + 0.5 * h10
    w_ext[j, seg + 3] += 0.5 * h11

    w = w_ext[:, 1:17].copy()
    w[:, 0] += 2 * w_ext[:, 0]
    w[:, 1] -= w_ext[:, 0]
    w[:, 15] += 2 * w_ext[:, 17]
    w[:, 14] -= w_ext[:, 17]
    return w.astype(np.float32)


def _w2rep() -> np.ndarray:
    """[128, 512]: W2[k*2+c, j*2+c] = W[j, k], replicated on 4 row-groups."""
    w = _spline_weights()
    w2 = np.zeros((32, 512), dtype=np.float32)
    jj = np.arange(T_OUT)
    for c in range(2):
        for k in range(N_CP):
            w2[k * 2 + c, jj * 2 + c] = w[jj, k]
    return np.tile(w2, (4, 1))


_W2REP = _w2rep()
_NC_CACHE = None


def _build():
    nc = bacc.Bacc(
        "TRN2", target_bir_lowering=False, debug=False, num_devices=N_CORES
    )
    f32 = mybir.dt.float32
    cp = nc.dram_tensor("cp", [B_SHARD, N_CP, 2], f32, kind="ExternalInput").ap()
    w2 = nc.dram_tensor("w2", [128, 512], f32, kind="ExternalInput").ap()
    out = nc.dram_tensor("out", [B_SHARD, T_OUT, 2], f32, kind="ExternalOutput").ap()

    # Batch mapping: batch = 512 g + 4 m + a, where a = PE row-group and
    # m = output psum partition. Then each output partition holds 4
    # consecutive batches across the a-staged free dim -> the output DMA of a
    # group is one flat [128, 2048] transfer (8 KB contiguous per partition).
    # The transpose needs tin[32a+v, 32u+kc] = cp[512g + 128u + 4v + a, kc],
    # loaded as 4 DMAs (one per row-group a).
    cp_v = cp.rearrange("(g u v a) k c -> g a v u (k c)", u=4, v=32, a=4)
    out_v = out.rearrange("(g p a) j c -> g p a (j c)", p=128, a=4)

    with TileContext(nc) as tc:
        with (
            tc.tile_pool(name="const", bufs=1) as cpool,
            tc.tile_pool(name="io", bufs=4) as io,
            tc.tile_pool(name="stage", bufs=3) as stg,
            tc.tile_pool(name="psum", bufs=8, space="PSUM") as pp,
        ):
            w2t = cpool.tile([128, 512], f32)
            nc.sync.dma_start(out=w2t[:], in_=w2[:])
            for g in range(GROUPS):
                tin = io.tile([128, 4, 32], f32, tag="tin")
                for a in range(4):
                    nc.sync.dma_start(
                        out=tin[32 * a : 32 * (a + 1), :, :], in_=cp_v[g, a]
                    )
                tt = io.tile([128, 128], f32, tag="tt")
                nc.vector.transpose(out=tt[:], in_=tin[:].rearrange("p q x -> p (q x)"))
                stage = stg.tile([128, 4, 512], f32, tag="stage")
                for a in range(4):
                    ps = pp.tile([128, 512], f32, tag="ps")
                    nc.tensor.matmul(
                        ps[:],
                        lhsT=tt[32 * a : 32 * (a + 1), :],
                        rhs=w2t[32 * a : 32 * (a + 1), :],
                        start=True,
                        stop=True,
                        tile_position=(32 * a, 0),
                    )
                    if a < 2:
                        nc.vector.tensor_copy(out=stage[:, a, :], in_=ps[:])
                    else:
                        nc.scalar.copy(out=stage[:, a, :], in_=ps[:])
                nc.sync.dma_start(out=out_v[g], in_=stage[:])
    nc.compile()
    return nc


def get_nc():
    global _NC_CACHE
    if _NC_CACHE is None:
        _NC_CACHE = _build()
    return _NC_CACHE


def kernel(control_points, num_output_points=None, **_unused):
    assert num_output_points is None or int(num_output_points) == T_OUT
    cp = np.ascontiguousarray(np.asarray(control_points, dtype=np.float32))
    assert cp.shape == (B_TOTAL, N_CP, 2), cp.shape

    nc = get_nc()
    shards = cp.reshape(N_CORES, B_SHARD, N_CP, 2)
    in_maps = [{"cp": shards[i], "w2": _W2REP} for i in range(N_CORES)]
    res = run_bass_kernel_spmd(nc, in_maps, core_ids=list(range(N_CORES)))
    return np.concatenate([res.results[i]["out"] for i in range(N_CORES)], axis=0)


# revision 6
# speedup vs baseline: 1.2986x; 1.2986x over previous
"""Trainium2 Bass kernel for nn_DifferentiableBSpline (Catmull-Rom spline eval).

The reference maps control_points [B, 16, 2] -> trajectory [B, 256, 2] where,
for the fixed schedule (n_cp=16, num_output_points=256), every output point is
a fixed linear combination of the 16 control points of its sample:

    out[b, j, c] = sum_k W[j, k] * cp[b, k, c]

with W[256, 16] folding the Hermite basis, the per-segment t schedule and the
boundary mirroring. On device this is a tiny-K batched matmul, memory bound on
the 128 MB output.

Per-core layout (pure data parallel over batch, B_shard = 8192):
  - groups of 512 batches; input tile [128, (q=4, kc=32)] loaded contiguously
  - DVE StreamTranspose (32x32 blocks) turns it into 4 row-groups of
    lhsT = [kc=32, 128 batches] stacked on partitions
  - 4 TensorE matmuls (K=32 row-tiled at partition 32a, M=128, N=512) against
    a replicated constant W2 [128, 512] -> psum [128, 512] each
  - PSUM drained to SBUF by DVE (a=0,1) and ACT (a=2,3), then one 1 MiB DMA
    writes 512 batches of output.
"""

import numpy as np

import concourse.mybir as mybir
from concourse import bacc
from concourse.tile import TileContext
from concourse.bass_utils import run_bass_kernel_spmd

N_CORES = 8
B_TOTAL = 65536
B_SHARD = B_TOTAL // N_CORES  # 8192
N_CP = 16
T_OUT = 256
GROUP_B = 512
GROUPS = B_SHARD // GROUP_B  # 16


def _spline_weights() -> np.ndarray:
    """W[256, 16]: trajectory[b] = W @ cp[b] (per coordinate)."""
    segments = N_CP - 1
    pps = T_OUT // segments + 1
    seg_list, t_list = [], []
    count = 0
    for i in range(segments):
        if i == segments - 1:
            ts = np.linspace(0.0, 1.0, T_OUT - count)
        else:
            ts = np.linspace(0.0, 1.0, pps)[:-1]
        seg_list.append(np.full(ts.shape, i, dtype=np.int64))
        t_list.append(ts)
        count += len(ts)
    seg = np.concatenate(seg_list)
    t = np.concatenate(t_list).astype(np.float32)
    assert len(seg) == T_OUT

    t2, t3 = t * t, t * t * t
    h00 = 2 * t3 - 3 * t2 + 1
    h10 = t3 - 2 * t2 + t
    h01 = -2 * t3 + 3 * t2
    h11 = t3 - t2

    j = np.arange(T_OUT)
    w_ext = np.zeros((T_OUT, N_CP + 2), dtype=np.float64)
    w_ext[j, seg] += -0.5 * h10
    w_ext[j, seg + 1] += h00 - 0.5 * h11
    w_ext[j, seg + 2] += h01 + 0.5 * h10
    w_ext[j, seg + 3] += 0.5 * h11

    w = w_ext[:, 1:17].copy()
    w[:, 0] += 2 * w_ext[:, 0]
    w[:, 1] -= w_ext[:, 0]
    w[:, 15] += 2 * w_ext[:, 17]
    w[:, 14] -= w_ext[:, 17]
    return w.astype(np.float32)


def _w2rep() -> np.ndarray:
    """[128, 512]: W2[k*2+c, j*2+c] = W[j, k], replicated on 4 row-groups."""
    w = _spline_weights()
    w2 = np.zeros((32, 512), dtype=np.float32)
    jj = np.arange(T_OUT)
    for c in range(2):
        for k in range(N_CP):
            w2[k * 2 + c, jj * 2 + c] = w[jj, k]
    return np.tile(w2, (4, 1))


_W2REP = _w2rep()
_NC_CACHE = None


def _build():
    nc = bacc.Bacc(
        "TRN2", target_bir_lowering=False, debug=False, num_devices=N_CORES
    )
    f32 = mybir.dt.float32
    cp = nc.dram_tensor("cp", [B_SHARD, N_CP, 2], f32, kind="ExternalInput").ap()
    w2 = nc.dram_tensor("w2", [128, 512], f32, kind="ExternalInput").ap()
    out = nc.dram_tensor("out", [B_SHARD, T_OUT, 2], f32, kind="ExternalOutput").ap()

    # Batch mapping: batch = 512 g + 4 m + a, where a = PE row-group and
    # m = output psum partition. Then each output partition holds 4
    # consecutive batches across the a-staged free dim -> the output DMA of a
    # group is one flat [128, 2048] transfer (8 KB contiguous per partition).
    # The transpose needs tin[32a+v, 32u+kc] = cp[512g + 128u + 4v + a, kc].
    # Inputs load on the SWDGE (gpsimd) queue so their many 128 B
    # descriptors never block the output stream on the HWDGE queue, batched
    # as HALVES x 4 row-group DMAs of [v=32, (g u), kc=32].
    HALVES = 2
    GPH = GROUPS // HALVES  # groups per half
    cp_v = cp.rearrange(
        "(h g u v a) k c -> h a v (g u) (k c)", h=HALVES, u=4, v=32, a=4
    )
    out_v = out.rearrange("(g p a) j c -> g p a (j c)", p=128, a=4)

    with TileContext(nc) as tc:
        with (
            tc.tile_pool(name="const", bufs=1) as cpool,
            tc.tile_pool(name="io", bufs=4) as io,
            tc.tile_pool(name="stage", bufs=3) as stg,
            tc.tile_pool(name="psum", bufs=8, space="PSUM") as pp,
        ):
            w2t = cpool.tile([128, 512], f32)
            nc.gpsimd.dma_start(out=w2t[:], in_=w2[:])
            halves = []
            for h in range(HALVES):
                tin = cpool.tile([128, GPH * 4, 32], f32, tag=f"tin{h}")
                for a in range(4):
                    nc.gpsimd.dma_start(
                        out=tin[32 * a : 32 * (a + 1), :, :], in_=cp_v[h, a]
                    )
                halves.append(tin)
            for g in range(GROUPS):
                tin = halves[g // GPH]
                gl = g % GPH
                tt = io.tile([128, 128], f32, tag="tt")
                nc.vector.transpose(
                    out=tt[:],
                    in_=tin[:, 4 * gl : 4 * (gl + 1), :].rearrange("p q x -> p (q x)"),
                )
                stage = stg.tile([128, 4, 512], f32, tag="stage")
                for a in range(4):
                    ps = pp.tile([128, 512], f32, tag="ps")
                    nc.tensor.matmul(
                        ps[:],
                        lhsT=tt[32 * a : 32 * (a + 1), :],
                        rhs=w2t[32 * a : 32 * (a + 1), :],
                        start=True,
                        stop=True,
                        tile_position=(32 * a, 0),
                    )
                    if a < 2:
                        nc.vector.tensor_copy(out=stage[:, a, :], in_=ps[:])
                    else:
                        nc.scalar.copy(out=stage[:, a, :], in_=ps[:])
                nc.sync.dma_start(out=out_v[g], in_=stage[:])
    nc.compile()
    return nc


def get_nc():
    global _NC_CACHE
    if _NC_CACHE is None:
        _NC_CACHE = _build()
    return _NC_CACHE


def kernel(control_points, num_output_points=None, **_unused):
    assert num_output_points is None or int(num_output_points) == T_OUT
    cp = np.ascontiguousarray(np.asarray(control_points, dtype=np.float32))
    assert cp.shape == (B_TOTAL, N_CP, 2), cp.shape

    nc = get_nc()
    shards = cp.reshape(N_CORES, B_SHARD, N_CP, 2)
    in_maps = [{"cp": shards[i], "w2": _W2REP} for i in range(N_CORES)]
    res = run_bass_kernel_spmd(nc, in_maps, core_ids=list(range(N_CORES)))
    return np.concatenate([res.results[i]["out"] for i in range(N_CORES)], axis=0)


# revision 15
# speedup vs baseline: 1.5085x; 1.1617x over previous
"""Trainium2 Bass kernel for nn_DifferentiableBSpline (Catmull-Rom spline eval).

The reference maps control_points [B, 16, 2] -> trajectory [B, 256, 2] where,
for the fixed schedule (n_cp=16, num_output_points=256), every output point is
a fixed linear combination of the 16 control points of its sample:

    out[b, j, c] = sum_k W[j, k] * cp[b, k, c]

with W[256, 16] folding the Hermite basis, the per-segment t schedule and the
boundary mirroring. On device this is a tiny-K batched matmul, memory bound on
the 128 MB output.

Per-core layout (pure data parallel over batch, B_shard = 8192):
  - groups of 512 batches; input tile [128, (q=4, kc=32)] loaded contiguously
  - DVE StreamTranspose (32x32 blocks) turns it into 4 row-groups of
    lhsT = [kc=32, 128 batches] stacked on partitions
  - 4 TensorE matmuls (K=32 row-tiled at partition 32a, M=128, N=512) against
    a replicated constant W2 [128, 512] -> psum [128, 512] each
  - PSUM drained to SBUF by DVE (a=0,1) and ACT (a=2,3), then one 1 MiB DMA
    writes 512 batches of output.
"""

import numpy as np

import concourse.mybir as mybir
from concourse import bacc
from concourse.tile import TileContext
from concourse.bass_utils import run_bass_kernel_spmd

N_CORES = 8
B_TOTAL = 65536
B_SHARD = B_TOTAL // N_CORES  # 8192
N_CP = 16
T_OUT = 256
GROUP_B = 512
GROUPS = B_SHARD // GROUP_B  # 16


def _spline_weights() -> np.ndarray:
    """W[256, 16]: trajectory[b] = W @ cp[b] (per coordinate)."""
    segments = N_CP - 1
    pps = T_OUT // segments + 1
    seg_list, t_list = [], []
    count = 0
    for i in range(segments):
        if i == segments - 1:
            ts = np.linspace(0.0, 1.0, T_OUT - count)
        else:
            ts = np.linspace(0.0, 1.0, pps)[:-1]
        seg_list.append(np.full(ts.shape, i, dtype=np.int64))
        t_list.append(ts)
        count += len(ts)
    seg = np.concatenate(seg_list)
    t = np.concatenate(t_list).astype(np.float32)
    assert len(seg) == T_OUT

    t2, t3 = t * t, t * t * t
    h00 = 2 * t3 - 3 * t2 + 1
    h10 = t3 - 2 * t2 + t
    h01 = -2 * t3 + 3 * t2
    h11 = t3 - t2

    j = np.arange(T_OUT)
    w_ext = np.zeros((T_OUT, N_CP + 2), dtype=np.float64)
    w_ext[j, seg] += -0.5 * h10
    w_ext[j, seg + 1] += h00 - 0.5 * h11
    w_ext[j, seg + 2] += h01 + 0.5 * h10
    w_ext[j, seg + 3] += 0.5 * h11

    w = w_ext[:, 1:17].copy()
    w[:, 0] += 2 * w_ext[:, 0]
    w[:, 1] -= w_ext[:, 0]
    w[:, 15] += 2 * w_ext[:, 17]
    w[:, 14] -= w_ext[:, 17]
    return w.astype(np.float32)


def _w2rep() -> np.ndarray:
    """[128, 512]: W2[k*2+c, j*2+c] = W[j, k], replicated on 4 row-groups."""
    w = _spline_weights()
    w2 = np.zeros((32, 512), dtype=np.float32)
    jj = np.arange(T_OUT)
    for c in range(2):
        for k in range(N_CP):
            w2[k * 2 + c, jj * 2 + c] = w[jj, k]
    return np.tile(w2, (4, 1))


_W2REP = _w2rep()
_NC_CACHE = None


def _build():
    nc = bacc.Bacc(
        "TRN2", target_bir_lowering=False, debug=False, num_devices=N_CORES
    )
    f32 = mybir.dt.float32
    f32r = mybir.dt.float32r
    cp = nc.dram_tensor("cp", [B_SHARD, N_CP, 2], f32, kind="ExternalInput").ap()
    w2 = nc.dram_tensor("w2", [128, 512], f32, kind="ExternalInput").ap()
    out = nc.dram_tensor("out", [B_SHARD, T_OUT, 2], f32, kind="ExternalOutput").ap()

    # Batch mapping: batch = 512 g + 4 m + a, where a = PE row-group and
    # m = output psum partition. Then each output partition holds 4
    # consecutive batches across the a-staged free dim -> the output DMA of a
    # group is one flat [128, 2048] transfer (8 KB contiguous per partition).
    # The transpose needs tin[32a+v, 32u+kc] = cp[512g + 128u + 4v + a, kc].
    # Inputs load in ramped chunks of groups: the first tiny chunks issue from
    # the idle SP engine so group 0 computes ~1.5us in; the bulk goes through
    # the SWDGE (gpsimd) queue so its many 128 B descriptors never block the
    # output stream on the HWDGE queue.
    CHUNKS = [(0, 1), (1, 2), (2, 4), (4, 8), (8, 16)]
    cp_g = cp.rearrange("(g u v a) k c -> g a v u (k c)", u=4, v=32, a=4)
    out_v = out.rearrange("(g p a) j c -> g p a (j c)", p=128, a=4)

    with TileContext(nc) as tc:
        with (
            tc.tile_pool(name="const", bufs=1) as cpool,
            tc.tile_pool(name="io", bufs=4) as io,
            tc.tile_pool(name="stage", bufs=3) as stg,
            tc.tile_pool(name="psum", bufs=8, space="PSUM") as pp,
        ):
            cp_va = cp.rearrange("(g u v a) k c -> a v (g u) (k c)", u=4, v=32, a=4)
            w2f = cpool.tile([128, 512], f32)
            nc.sync.dma_start(out=w2f[:], in_=w2[:])
            w2t = cpool.tile([128, 512], f32r)
            nc.vector.tensor_copy(out=w2t[:], in_=w2f[:])
            chunk_tiles = {}
            for ci, (g0, g1) in enumerate(CHUNKS):
                tin = cpool.tile([128, (g1 - g0) * 4, 32], f32, tag=f"tin{ci}")
                eng = nc.sync if ci < 2 else nc.gpsimd
                for a in range(4):
                    eng.dma_start(
                        out=tin[32 * a : 32 * (a + 1), :, :],
                        in_=cp_va[a, :, 4 * g0 : 4 * g1, :],
                    )
                for g in range(g0, g1):
                    chunk_tiles[g] = (tin, g - g0)
            for g in range(GROUPS):
                tin, gl = chunk_tiles[g]
                tt = io.tile([128, 128], f32, tag="tt")
                nc.vector.transpose(
                    out=tt[:],
                    in_=tin[:, 4 * gl : 4 * (gl + 1), :].rearrange("p q x -> p (q x)"),
                )
                ttr = io.tile([128, 128], f32r, tag="ttr")
                nc.vector.tensor_copy(out=ttr[:], in_=tt[:])
                stage = stg.tile([128, 4, 512], f32, tag="stage")
                for a in range(4):
                    ps = pp.tile([128, 512], f32, tag="ps")
                    nc.tensor.matmul(
                        ps[:],
                        lhsT=ttr[32 * a : 32 * (a + 1), :],
                        rhs=w2t[32 * a : 32 * (a + 1), :],
                        start=True,
                        stop=True,
                        tile_position=(32 * a, 0),
                    )
                    if a < 2:
                        nc.vector.tensor_copy(out=stage[:, a, :], in_=ps[:])
                    else:
                        nc.scalar.copy(out=stage[:, a, :], in_=ps[:])
                nc.sync.dma_start(out=out_v[g], in_=stage[:])
    nc.compile()
    return nc


def get_nc():
    global _NC_CACHE
    if _NC_CACHE is None:
        _NC_CACHE = _build()
    return _NC_CACHE


def kernel(control_points, num_output_points=None, **_unused):
    assert num_output_points is None or int(num_output_points) == T_OUT
    cp = np.ascontiguousarray(np.asarray(control_points, dtype=np.float32))
    assert cp.shape == (B_TOTAL, N_CP, 2), cp.shape

    nc = get_nc()
    shards = cp.reshape(N_CORES, B_SHARD, N_CP, 2)
    in_maps = [{"cp": shards[i], "w2": _W2REP} for i in range(N_CORES)]
    res = run_bass_kernel_spmd(nc, in_maps, core_ids=list(range(N_CORES)))
    return np.concatenate([res.results[i]["out"] for i in range(N_CORES)], axis=0)
